# revision 14
# baseline (speedup 1.0000x reference)
"""Haar wavelet (2x2 stride-2, per-channel) Trainium2 Bass kernel.

Full input x: (8, 64, 512, 512) f32 -> full output (8, 256, 256, 256) f32.
Sharding: pure data parallel over batch -- core i processes x[i].

I/O in fp16: the host casts x to fp16 (rel err ~5e-4, far inside the
2e-2 gate) and upcasts the fp16 result; device traffic drops 2x vs f32
(67 MB/core -> ~187 us at 358 GB/s).

Per-core layout (C=64 channels, H=W=512, KC=4 channels per block):
  - Block = KC channels. Rows of the KC channels are flattened and
    dealt to partitions 16-consecutive-rows each: partition p = 32*k+q
    holds rows [16q, 16q+16) of channel c0+k -- one 16 KB contiguous
    DRAM run per partition per load DMA (128 descriptors / 2 MB DMA).
  - Halve in place (DVE tensor_scalar, 4x fp16 mode).
  - Vertical butterfly (DVE tensor_tensor, 2x fp16 mode, unit stride):
      s = top + bot ; d = bot - top          (8 row-pairs / partition)
  - Horizontal butterfly (stride-2 reads, 1x): ll = s_e + s_o and
    lh = d_e + d_o on DVE; hl = s_o - s_e and hh = d_o - d_e on
    GpSimd (Pool) so DVE stays under the DMA roofline.
  - Store: partition p writes 4 output channels x 8 consecutive rows
    x 256 -- 4 KB contiguous DRAM runs (512 descriptors / 2 MB DMA).
Engine budget per core: DMA ~187 us (bound), DVE ~170 us, Pool ~130 us,
ACT = load ring, SP = store ring.
"""

import sys

if "/opt/trn_rl_repo" not in sys.path:
    sys.path.insert(0, "/opt/trn_rl_repo")

from contextlib import ExitStack

import numpy as np

import concourse.bass as bass
import concourse.tile as tile
from concourse import bacc
from concourse import mybir
from concourse.bass_utils import run_bass_kernel_spmd

N_CORES = 8
C, H, W = 64, 512, 512
F16 = mybir.dt.float16
ADD = mybir.AluOpType.add
SUB = mybir.AluOpType.subtract

_CACHED = {}


def _build(C=C, H=H, W=W, KC=4, n_pool_ops=1, halve_on="act"):
    HO, WO = H // 2, W // 2
    RP = 4 * KC          # input rows per partition (16 for KC=4)
    M = RP // 2          # output rows per partition (8)
    PPC = 128 // KC      # partitions per channel (32)
    assert H % RP == 0 and PPC * RP == H
    nc = bacc.Bacc("TRN2", target_bir_lowering=False, debug=False)
    x = nc.dram_tensor("x", [C, H, W], F16, kind="ExternalInput").ap()
    # Device-order output: [cg, k, q, c, m, wo] flattened -- exactly the SBUF
    # store order, so each block is ONE DMA with 16 KB contiguous runs. The
    # host permutes to [4C, HO, WO] afterwards.
    out = nc.dram_tensor(
        "out", [C // KC, 128, 4 * M * WO], F16, kind="ExternalOutput"
    ).ap()

    with tile.TileContext(nc) as tc, ExitStack() as ctx:
        xpool = ctx.enter_context(tc.tile_pool(name="xp", bufs=3))
        mpool = ctx.enter_context(tc.tile_pool(name="mid", bufs=3))
        rpool = ctx.enter_context(tc.tile_pool(name="raw", bufs=3))

        pending = None  # (s_t, d_t, c0) awaiting stage2 + store

        def stage2_and_store(s_t, d_t, c0):
            # horizontal butterfly (stride-2 reads, flat APs): DVE + Pool
            s2 = s_t[:].rearrange("p (we e) -> p we e", e=2)
            d2 = d_t[:].rearrange("p (we e) -> p we e", e=2)
            s_e, s_o = s2[:, :, 0], s2[:, :, 1]
            d_e, d_o = d2[:, :, 0], d2[:, :, 1]

            rt = rpool.tile([128, 4 * M * WO], F16)
            r4 = rt[:].rearrange("p (c mwo) -> p c mwo", c=4)
            ops = [
                (r4[:, 0], s_e, s_o, ADD),  # ll
                (r4[:, 1], d_e, d_o, ADD),  # lh
                (r4[:, 2], s_o, s_e, SUB),  # hl
                (r4[:, 3], d_o, d_e, SUB),  # hh
            ]
            for i, (dst_v, a, b, op) in enumerate(ops):
                eng = nc.gpsimd if i >= 4 - n_pool_ops else nc.vector
                eng.tensor_tensor(dst_v, a, b, op)

            # store: one DMA per block into device-order out; partition p's
            # 16 KB row is contiguous in DRAM.
            nc.sync.dma_start(out[c0 // KC], rt[:])

        for c0 in range(0, C, KC):
            # ---- load: partition 32*k+q <- rows [16q, 16q+16) of chan c0+k
            xt = xpool.tile([128, RP * W], F16)
            src = x[c0 : c0 + KC, :, :].rearrange(
                "k (q t) w -> (k q) (t w)", t=RP
            )
            nc.scalar.dma_start(xt[:], src)

            # ---- halve (fp16 4x mode on DVE, or on ACT)
            if halve_on == "dve":
                nc.vector.tensor_scalar_mul(xt[:], xt[:], 0.5)
            else:
                nc.scalar.mul(xt[:], xt[:], 0.5)

            # ---- vertical butterfly (DVE, unit stride, fp16 2x)
            x4 = xt[:].rearrange("p (m t w) -> p m t w", m=M, t=2)
            top, bot = x4[:, :, 0, :], x4[:, :, 1, :]
            s_t = mpool.tile([128, M * W], F16)
            d_t = mpool.tile([128, M * W], F16)
            sv = s_t[:].rearrange("p (m w) -> p m w", m=M)
            dv = d_t[:].rearrange("p (m w) -> p m w", m=M)
            nc.vector.tensor_tensor(sv, top, bot, ADD)
            nc.vector.tensor_tensor(dv, bot, top, SUB)

            # ---- previous block's stage2 + store (hides SBUF write-ack
            # latency of s/d behind the next block's stage1)
            if pending is not None:
                stage2_and_store(*pending)
            pending = (s_t, d_t, c0)

        stage2_and_store(*pending)
    nc.compile()
    return nc


def _get_nc():
    if "nc" not in _CACHED:
        _CACHED["nc"] = _build()
    return _CACHED["nc"]


def _run(x, **kwargs):
    x = np.asarray(x)
    assert x.shape == (N_CORES, C, H, W), x.shape
    x16 = np.ascontiguousarray(x).astype(np.float16)
    nc = _get_nc()
    in_maps = [{"x": x16[i]} for i in range(N_CORES)]
    res = run_bass_kernel_spmd(nc, in_maps, core_ids=list(range(N_CORES)), **kwargs)
    out = np.stack([res.results[i]["out"] for i in range(N_CORES)], axis=0)
    # device order [cg, (k q), (c m wo)] -> [4C, HO, WO]
    KC, M = 4, 8
    out = out.reshape(N_CORES, C // KC, KC, 128 // KC, 4, M, W // 2)
    out = out.transpose(0, 1, 2, 4, 3, 5, 6).reshape(N_CORES, 4 * C, H // 2, W // 2)
    return np.ascontiguousarray(out).astype(np.float32), res


def kernel(x):
    return _run(x)[0]


# revision 15
# speedup vs baseline: 1.0758x; 1.0758x over previous
"""Haar wavelet (2x2 stride-2, per-channel) Trainium2 Bass kernel.

Full input x: (8, 64, 512, 512) f32 -> full output (8, 256, 256, 256) f32.
Sharding: pure data parallel over batch -- core i processes x[i].

I/O in fp16: the host casts x to fp16 (rel err ~5e-4, far inside the
2e-2 gate) and upcasts the fp16 result; device traffic drops 2x vs f32
(67 MB/core -> ~187 us at 358 GB/s).

Per-core layout (C=64 channels, H=W=512, KC=4 channels per block):
  - Block = KC channels. Rows of the KC channels are flattened and
    dealt to partitions 16-consecutive-rows each: partition p = 32*k+q
    holds rows [16q, 16q+16) of channel c0+k -- one 16 KB contiguous
    DRAM run per partition per load DMA (128 descriptors / 2 MB DMA).
  - Halve in place (DVE tensor_scalar, 4x fp16 mode).
  - Vertical butterfly (DVE tensor_tensor, 2x fp16 mode, unit stride):
      s = top + bot ; d = bot - top          (8 row-pairs / partition)
  - Horizontal butterfly (stride-2 reads, 1x): ll = s_e + s_o and
    lh = d_e + d_o on DVE; hl = s_o - s_e and hh = d_o - d_e on
    GpSimd (Pool) so DVE stays under the DMA roofline.
  - Store: partition p writes 4 output channels x 8 consecutive rows
    x 256 -- 4 KB contiguous DRAM runs (512 descriptors / 2 MB DMA).
Engine budget per core: DMA ~187 us (bound), DVE ~170 us, Pool ~130 us,
ACT = load ring, SP = store ring.
"""

import sys

if "/opt/trn_rl_repo" not in sys.path:
    sys.path.insert(0, "/opt/trn_rl_repo")

from contextlib import ExitStack

import numpy as np

import concourse.bass as bass
import concourse.tile as tile
from concourse import bacc
from concourse import mybir
from concourse.bass_utils import run_bass_kernel_spmd

N_CORES = 8
C, H, W = 64, 512, 512
F16 = mybir.dt.float16
ADD = mybir.AluOpType.add
SUB = mybir.AluOpType.subtract

_CACHED = {}


def _build(C=C, H=H, W=W, KC=4, n_pool_ops=0, halve_on="act"):
    HO, WO = H // 2, W // 2
    RP = 4 * KC          # input rows per partition (16 for KC=4)
    M = RP // 2          # output rows per partition (8)
    PPC = 128 // KC      # partitions per channel (32)
    assert H % RP == 0 and PPC * RP == H
    nc = bacc.Bacc("TRN2", target_bir_lowering=False, debug=False)
    x = nc.dram_tensor("x", [C, H, W], F16, kind="ExternalInput").ap()
    # Device-order output: [cg, k, q, c, m, wo] flattened -- exactly the SBUF
    # store order, so each block is ONE DMA with 16 KB contiguous runs. The
    # host permutes to [4C, HO, WO] afterwards.
    out = nc.dram_tensor(
        "out", [C // KC, 128, 4 * M * WO], F16, kind="ExternalOutput"
    ).ap()

    with tile.TileContext(nc) as tc, ExitStack() as ctx:
        xpool = ctx.enter_context(tc.tile_pool(name="xp", bufs=3))
        mpool = ctx.enter_context(tc.tile_pool(name="mid", bufs=3))
        rpool = ctx.enter_context(tc.tile_pool(name="raw", bufs=3))

        pending = None  # (s_t, d_t, c0) awaiting stage2 + store

        def stage2_and_store(s_t, d_t, c0):
            # horizontal butterfly (stride-2 reads, flat APs): DVE + Pool
            s2 = s_t[:].rearrange("p (we e) -> p we e", e=2)
            d2 = d_t[:].rearrange("p (we e) -> p we e", e=2)
            s_e, s_o = s2[:, :, 0], s2[:, :, 1]
            d_e, d_o = d2[:, :, 0], d2[:, :, 1]

            rt = rpool.tile([128, 4 * M * WO], F16)
            r4 = rt[:].rearrange("p (c mwo) -> p c mwo", c=4)
            ops = [
                (r4[:, 0], s_e, s_o, ADD),  # ll
                (r4[:, 1], d_e, d_o, ADD),  # lh
                (r4[:, 2], s_o, s_e, SUB),  # hl
                (r4[:, 3], d_o, d_e, SUB),  # hh
            ]
            for i, (dst_v, a, b, op) in enumerate(ops):
                eng = nc.gpsimd if i >= 4 - n_pool_ops else nc.vector
                eng.tensor_tensor(dst_v, a, b, op)

            # store: one DMA per block into device-order out; partition p's
            # 16 KB row is contiguous in DRAM.
            nc.sync.dma_start(out[c0 // KC], rt[:])

        for c0 in range(0, C, KC):
            # ---- load: partition 32*k+q <- rows [16q, 16q+16) of chan c0+k
            xt = xpool.tile([128, RP * W], F16)
            src = x[c0 : c0 + KC, :, :].rearrange(
                "k (q t) w -> (k q) (t w)", t=RP
            )
            nc.scalar.dma_start(xt[:], src)

            # ---- halve (fp16 4x mode on DVE, or on ACT)
            if halve_on == "dve":
                nc.vector.tensor_scalar_mul(xt[:], xt[:], 0.5)
            else:
                nc.scalar.mul(xt[:], xt[:], 0.5)

            # ---- vertical butterfly (DVE, unit stride, fp16 2x)
            x4 = xt[:].rearrange("p (m t w) -> p m t w", m=M, t=2)
            top, bot = x4[:, :, 0, :], x4[:, :, 1, :]
            s_t = mpool.tile([128, M * W], F16)
            d_t = mpool.tile([128, M * W], F16)
            sv = s_t[:].rearrange("p (m w) -> p m w", m=M)
            dv = d_t[:].rearrange("p (m w) -> p m w", m=M)
            nc.vector.tensor_tensor(sv, top, bot, ADD)
            nc.vector.tensor_tensor(dv, bot, top, SUB)

            # ---- previous block's stage2 + store (hides SBUF write-ack
            # latency of s/d behind the next block's stage1)
            if pending is not None:
                stage2_and_store(*pending)
            pending = (s_t, d_t, c0)

        stage2_and_store(*pending)
    nc.compile()
    return nc


def _get_nc():
    if "nc" not in _CACHED:
        _CACHED["nc"] = _build()
    return _CACHED["nc"]


def _run(x, **kwargs):
    x = np.asarray(x)
    assert x.shape == (N_CORES, C, H, W), x.shape
    x16 = np.ascontiguousarray(x).astype(np.float16)
    nc = _get_nc()
    in_maps = [{"x": x16[i]} for i in range(N_CORES)]
    res = run_bass_kernel_spmd(nc, in_maps, core_ids=list(range(N_CORES)), **kwargs)
    out = np.stack([res.results[i]["out"] for i in range(N_CORES)], axis=0)
    # device order [cg, (k q), (c m wo)] -> [4C, HO, WO]
    KC, M = 4, 8
    out = out.reshape(N_CORES, C // KC, KC, 128 // KC, 4, M, W // 2)
    out = out.transpose(0, 1, 2, 4, 3, 5, 6).reshape(N_CORES, 4 * C, H // 2, W // 2)
    return np.ascontiguousarray(out).astype(np.float32), res


def kernel(x):
    return _run(x)[0]


# revision 16
# speedup vs baseline: 1.4060x; 1.3070x over previous
"""Haar wavelet (2x2 stride-2, per-channel) Trainium2 Bass kernel.

Full input x: (8, 64, 512, 512) f32 -> full output (8, 256, 256, 256) f32.
Sharding: pure data parallel over batch -- core i processes x[i].

I/O in fp16: the host casts x to fp16 (rel err ~8e-4, far inside the
2e-2 gate) and upcasts the fp16 result; device traffic drops 2x vs f32
(67 MB/core -> ~187 us at 358 GB/s). The output DRAM tensor is laid out
in device store order (one 2 MB DMA per block, 16 KB contiguous runs);
the host permutes to the logical [4C, H/2, W/2] layout.

Per-core layout (C=64 channels, H=W=512, KC=4 channels per block):
  - Block = KC channels. Rows flattened and dealt 16-consecutive-rows
    per partition: partition 32k+q holds rows [16q, 16q+16) of channel
    c0+k -- one 16 KB contiguous DRAM run per partition per load.
  - ACT (scalar engine): deinterleave + halve fused: xeh = 0.5*x[even w],
    xoh = 0.5*x[odd w] (strided reads run at full ACT rate; this is the
    ONLY strided work, moved off the critical DVE engine).
  - DVE: horizontal butterfly A = xeh+xoh, B = xoh-xeh (packed, fp16 2x)
    then vertical butterfly ll = A0+A1, lh = A1-A0, hl = B0+B1,
    hh = B1-B0 (packed, 2x). All DVE ops run in fast 2x mode.
  - GpSimd stays idle: concurrent strided work on two engines contends
    for SBUF bandwidth and makes both ~2.4x slower (measured).
Engine budget per core: DMA ~188 us (bound), DVE ~150 us, ACT ~130 us.
"""

import sys

if "/opt/trn_rl_repo" not in sys.path:
    sys.path.insert(0, "/opt/trn_rl_repo")

from contextlib import ExitStack

import numpy as np

import concourse.bass as bass
import concourse.tile as tile
from concourse import bacc
from concourse import mybir
from concourse.bass_utils import run_bass_kernel_spmd

N_CORES = 8
C, H, W = 64, 512, 512
F16 = mybir.dt.float16
ADD = mybir.AluOpType.add
SUB = mybir.AluOpType.subtract

_CACHED = {}


def _build(C=C, H=H, W=W, KC=4):
    HO, WO = H // 2, W // 2
    RP = 4 * KC          # input rows per partition (16)
    M = RP // 2          # output rows per partition (8)
    PPC = 128 // KC      # partitions per channel (32)
    assert H % RP == 0 and PPC * RP == H
    nc = bacc.Bacc("TRN2", target_bir_lowering=False, debug=False)
    x = nc.dram_tensor("x", [C, H, W], F16, kind="ExternalInput").ap()
    # Device-order output: [cg, (k q), (band m wo)] -- exactly the SBUF
    # store order. Host permutes to [4C, HO, WO].
    out = nc.dram_tensor(
        "out", [C // KC, 128, 4 * M * WO], F16, kind="ExternalOutput"
    ).ap()

    with tile.TileContext(nc) as tc, ExitStack() as ctx:
        xpool = ctx.enter_context(tc.tile_pool(name="xp", bufs=3))
        epool = ctx.enter_context(tc.tile_pool(name="eo", bufs=2))
        apool = ctx.enter_context(tc.tile_pool(name="ab", bufs=2))
        rpool = ctx.enter_context(tc.tile_pool(name="raw", bufs=3))

        for c0 in range(0, C, KC):
            # ---- load: partition 32k+q <- rows [16q, 16q+16) of chan c0+k
            xt = xpool.tile([128, RP * W], F16)
            src = x[c0 : c0 + KC, :, :].rearrange(
                "k (q t) w -> (k q) (t w)", t=RP
            )
            nc.scalar.dma_start(xt[:], src)

            # ---- ACT: fused deinterleave + halve (strided reads)
            xf = xt[:].rearrange("p (we e) -> p we e", e=2)
            xeh = epool.tile([128, RP * WO], F16)
            xoh = epool.tile([128, RP * WO], F16)
            nc.scalar.mul(xeh[:], xf[:, :, 0], 0.5)
            nc.scalar.mul(xoh[:], xf[:, :, 1], 0.5)

            # ---- DVE: horizontal butterfly (packed, fp16 2x)
            at = apool.tile([128, RP * WO], F16)
            bt = apool.tile([128, RP * WO], F16)
            nc.vector.tensor_tensor(at[:], xeh[:], xoh[:], ADD)
            nc.vector.tensor_tensor(bt[:], xoh[:], xeh[:], SUB)

            # ---- DVE: vertical butterfly (packed, fp16 2x)
            a4 = at[:].rearrange("p (m t wo) -> p m t wo", m=M, t=2)
            b4 = bt[:].rearrange("p (m t wo) -> p m t wo", m=M, t=2)
            a0, a1 = a4[:, :, 0, :], a4[:, :, 1, :]
            b0, b1 = b4[:, :, 0, :], b4[:, :, 1, :]

            rt = rpool.tile([128, 4 * M * WO], F16)
            r4 = rt[:].rearrange("p (c m wo) -> p c m wo", c=4, m=M)
            nc.vector.tensor_tensor(r4[:, 0], a0, a1, ADD)  # ll
            nc.vector.tensor_tensor(r4[:, 1], a1, a0, SUB)  # lh
            nc.vector.tensor_tensor(r4[:, 2], b0, b1, ADD)  # hl
            nc.vector.tensor_tensor(r4[:, 3], b1, b0, SUB)  # hh

            # ---- store: one DMA per block; 16 KB contiguous runs
            nc.sync.dma_start(out[c0 // KC], rt[:])
    nc.compile()
    return nc


def _get_nc():
    if "nc" not in _CACHED:
        _CACHED["nc"] = _build()
    return _CACHED["nc"]


def _run(x, **kwargs):
    x = np.asarray(x)
    assert x.shape == (N_CORES, C, H, W), x.shape
    x16 = np.ascontiguousarray(x).astype(np.float16)
    nc = _get_nc()
    in_maps = [{"x": x16[i]} for i in range(N_CORES)]
    res = run_bass_kernel_spmd(nc, in_maps, core_ids=list(range(N_CORES)), **kwargs)
    out = np.stack([res.results[i]["out"] for i in range(N_CORES)], axis=0)
    # device order [cg, (k q), (band m wo)] -> [4C, HO, WO]
    KC, M = 4, 8
    out = out.reshape(N_CORES, C // KC, KC, 128 // KC, 4, M, W // 2)
    out = out.transpose(0, 1, 2, 4, 3, 5, 6).reshape(N_CORES, 4 * C, H // 2, W // 2)
    return np.ascontiguousarray(out).astype(np.float32), res


def kernel(x):
    return _run(x)[0]
